# revision 9
# baseline (speedup 1.0000x reference)
"""GNN ensemble MoE-routing kernel for Trainium2 (8 NeuronCores).

Reference computes all 8 expert MLPs for every sample then selects one
(8x wasted FLOPs). This kernel routes on the host instead: samples are
gathered per expert, and core c runs ONLY expert c's MLP over the
samples routed to it (expert-parallel sharding).

Math folding (exact):
  lat = eps*sigma_c + mu_c  =>  lat @ W1_c = eps @ (sigma_c*W1_c) + mu_c@W1_c
so the device computes  sigmoid(eps @ W1p + b1p) @ W2 + b2  with
  W1p = sigma_c * W1_c,  b1p = b1_c + mu_c @ W1_c  (folded on host).

Device layout: features on SBUF partitions, samples on the free axis.
  epsT [512, K]  ->  H^T [1024, K]  ->  Y^T [512, K]
All matmul operands are bf16 (PSUM accumulation stays fp32; rel-err
~3e-3 vs the 2e-2 gate): halves HBM traffic and enables FWL weight
loads. The host pre-interleaves every tensor into a [128, blocks, n]
layout so each loads with ONE dma_start (a dma_start costs ~0.6us
serialized on the Sync sequencer; the fp32r baseline spent ~12us just
issuing its 54 descriptors, and late x-chunk issues starved the PE
mid-kernel). All x chunks are prefetched up front; outputs go back in
one coalesced DMA per chunk. First chunk is 256 wide so the PE starts
on a small DMA dependency; the trailing chunk is the k_cap remainder
so the end-of-kernel bias-add + store tail is short.
"""

from contextlib import ExitStack

import numpy as np

import concourse.bass as bass
import concourse.tile as tile
from concourse import bacc, mybir
from concourse.bass_utils import run_bass_kernel_spmd

NB_COMP = 8
LAT_DIM = 512
NB_NEUR = 1024
OUT_DIM = 512
N_CORES = 8
KC1, MC1 = LAT_DIM // 128, NB_NEUR // 128  # 4, 8
KC2, MC2 = NB_NEUR // 128, OUT_DIM // 128  # 8, 4

F32 = mybir.dt.float32
BF16 = mybir.dt.bfloat16
NP_BF16 = mybir.dt.np(BF16)
SIG = mybir.ActivationFunctionType.Sigmoid
IDENT = mybir.ActivationFunctionType.Identity

_program_cache = {}


def _make_chunks(k_cap):
    # First chunk 256 (small DMA dependency -> PE starts early), then
    # 512-wide chunks; the remainder is emitted last in <=256 pieces so
    # the end-of-kernel bias-add + store + DMA-receipt tail is short
    # (extra chunks are nearly free: ~3ns/matmul dispatch overhead warm).
    chunks = []
    rem = k_cap
    first = min(256, rem)
    chunks.append(first)
    rem -= first
    while rem >= 512:
        chunks.append(512)
        rem -= 512
    while rem > 0:
        ns = min(256, rem)
        if rem > 256:
            ns = rem - 128
        chunks.append(ns)
        rem -= ns
    out = []
    n0 = 0
    for ns in chunks:
        out.append((n0, ns))
        n0 += ns
    return out


def _build_program(k_cap):
    """One-expert MLP over k_cap samples; same program runs SPMD on all 8 cores."""
    chunks = _make_chunks(k_cap)

    nc = bacc.Bacc(
        "TRN2",
        target_bir_lowering=False,
        debug=False,
        enable_asserts=False,
        num_devices=N_CORES,
    )
    # Host-packed layouts, partition dim first everywhere:
    #   x[p, j, n]  = epsT[j*128 + p, n]       (j = mm1 contraction block)
    #   w1[p, j, m] = W1p[j*128 + p, m]
    #   w2[p, j, o] = W2[j*128 + p, o]
    #   b1[p, mc]   = b1p[mc*128 + p],  b2[p, oc] = b2[oc*128 + p]
    #   y[p, oc, n] = yT[oc*128 + p, n]
    x_d = nc.dram_tensor("x", [128, KC1, k_cap], BF16, kind="ExternalInput").ap()
    w1_d = nc.dram_tensor("w1", [128, KC1, NB_NEUR], BF16, kind="ExternalInput").ap()
    b1_d = nc.dram_tensor("b1", [128, MC1], F32, kind="ExternalInput").ap()
    w2_d = nc.dram_tensor("w2", [128, KC2, OUT_DIM], BF16, kind="ExternalInput").ap()
    b2_d = nc.dram_tensor("b2", [128, MC2], F32, kind="ExternalInput").ap()
    y_d = nc.dram_tensor("y", [128, MC2, k_cap], BF16, kind="ExternalOutput").ap()

    with tile.TileContext(nc) as tc, ExitStack() as ctx:
        wpool = ctx.enter_context(tc.tile_pool(name="weights", bufs=1))
        xpool = ctx.enter_context(tc.tile_pool(name="x", bufs=1))
        hpool = ctx.enter_context(tc.tile_pool(name="h", bufs=1))
        ypool = ctx.enter_context(tc.tile_pool(name="y", bufs=2))
        # One shared pool holding all 8 PSUM banks; mm1 keeps 8 accumulators
        # live (kc-outer order), mm2 4, cycling through the same slots.
        pspool = ctx.enter_context(tc.tile_pool(name="ps", bufs=8, space="PSUM"))

        # Warm the PE's HAM clock gate while the first DMAs are in flight:
        # the gate needs ~3.4us of sustained PE activity to lift the 1.2GHz
        # cold throttle, so a dozen dummy matmuls on a memset tile make the
        # real matmuls run at 2.4GHz from their first instruction.
        warm = wpool.tile([128, 512], BF16, tag="warm")
        nc.gpsimd.memset(warm[:], 0.0)
        wps = pspool.tile([128, 512], F32, tag="ps", name="warm_ps")
        for _ in range(10):
            nc.tensor.matmul(wps[:], warm[:, :128], warm[:], start=True, stop=True)

        # Startup is DMA-supply-bound: one HWDGE ring delivers ~275-320 GB/s
        # with ~1.5us start latency and ~1.5us completion-receipt lag, which
        # is a hair slower than the warm PE's demand schedule. So the input
        # set is split across BOTH HWDGE rings (they round-robin the SDMA
        # engines): the weight stream goes on the scalar/ACT ring, the x
        # stream + w2 on the sync ring, ordered to match when the PE needs
        # each block. Output DMAs ride the ACT ring, which is idle after
        # startup, keeping them clear of the bulk x transfer.
        n0_0, ns_0 = chunks[0]
        w1a = wpool.tile([128, NB_NEUR], BF16, tag="w1a")
        nc.scalar.dma_start(w1a[:], w1_d[:, 0, :])
        xt = []
        t = xpool.tile([128, KC1, ns_0], BF16, tag="x0", name="x_0")
        nc.sync.dma_start(t[:], x_d[:, :, n0_0 : n0_0 + ns_0])
        xt.append(t)
        w1b = wpool.tile([128, NB_NEUR], BF16, tag="w1b")
        nc.scalar.dma_start(w1b[:], w1_d[:, 1, :])
        b1t = wpool.tile([128, MC1], F32, tag="b1")
        nc.scalar.dma_start(b1t[:], b1_d[:])
        w1c = wpool.tile([128, KC1 - 2, NB_NEUR], BF16, tag="w1c")
        nc.scalar.dma_start(w1c[:], w1_d[:, 2:, :])
        b2t = wpool.tile([128, MC2], F32, tag="b2")
        nc.scalar.dma_start(b2t[:], b2_d[:])
        if len(chunks) > 1:
            n0, ns = chunks[1]
            t = xpool.tile([128, KC1, ns], BF16, tag="x1", name="x_1")
            nc.sync.dma_start(t[:], x_d[:, :, n0 : n0 + ns])
            xt.append(t)
        w2t = wpool.tile([128, KC2, OUT_DIM], BF16, tag="w2")
        nc.sync.dma_start(w2t[:], w2_d[:, :, :])
        if len(chunks) > 2:
            n0r = chunks[2][0]
            nsr = k_cap - n0r
            xr = xpool.tile([128, KC1, nsr], BF16, tag="xr", name="x_rest")
            nc.sync.dma_start(xr[:], x_d[:, :, n0r:])
        else:
            xr = None

        def x_ap(ci, kc):
            n0, ns = chunks[ci]
            if ci < 2:
                return xt[ci][:, kc, :]
            return xr[:, kc, n0 - n0r : n0 - n0r + ns]

        def w1_ap(kc, mc):
            if kc == 0:
                return w1a[:, mc * 128 : (mc + 1) * 128]
            if kc == 1:
                return w1b[:, mc * 128 : (mc + 1) * 128]
            return w1c[:, kc - 2, mc * 128 : (mc + 1) * 128]

        for ci, (n0, ns) in enumerate(chunks):
            ht = []
            ps1 = [
                pspool.tile([128, ns], F32, tag="ps", name=f"ps1_{ci}_{i}")
                for i in range(MC1)
            ]
            for kc in range(KC1):
                for mc in range(MC1):
                    nc.tensor.matmul(
                        ps1[mc][:],
                        w1_ap(kc, mc),
                        x_ap(ci, kc),
                        start=(kc == 0),
                        stop=(kc == KC1 - 1),
                    )
                    if kc == KC1 - 1:
                        h = hpool.tile([128, ns], BF16, tag=f"h{mc}")
                        nc.scalar.activation(
                            h[:], ps1[mc][:], SIG, bias=b1t[:, mc : mc + 1]
                        )
                        ht.append(h)

            yt = ypool.tile([128, MC2, ns], BF16, tag="y", name=f"y_{ci}")
            ps2 = [
                pspool.tile([128, ns], F32, tag="ps", name=f"ps2_{ci}_{i}")
                for i in range(MC2)
            ]
            for kc in range(KC2):
                for oc in range(MC2):
                    nc.tensor.matmul(
                        ps2[oc][:],
                        w2t[:, kc, oc * 128 : (oc + 1) * 128],
                        ht[kc][:],
                        start=(kc == 0),
                        stop=(kc == KC2 - 1),
                    )
                    if kc == KC2 - 1:
                        # Alternate DVE / ACT so the 4 bias-adds of the last
                        # chunk drain on two engines instead of one.
                        if oc % 2 == 0:
                            nc.vector.tensor_scalar_add(
                                yt[:, oc, :], ps2[oc][:], b2t[:, oc : oc + 1]
                            )
                        else:
                            nc.scalar.activation(
                                yt[:, oc, :],
                                ps2[oc][:],
                                IDENT,
                                bias=b2t[:, oc : oc + 1],
                            )
            nc.scalar.dma_start(y_d[:, :, n0 : n0 + ns], yt[:])

    nc.compile()
    return nc


def get_program(k_cap):
    if k_cap not in _program_cache:
        _program_cache[k_cap] = _build_program(k_cap)
    return _program_cache[k_cap]


def _softplus(x):
    x = x.astype(np.float64)
    return np.maximum(x, 0.0) + np.log1p(np.exp(-np.abs(x)))


def kernel(epsilon, comp_idx, mu, rho, W1, b1, W2, b2, _trace=False):
    epsilon = np.asarray(epsilon, dtype=np.float32)
    comp_idx = np.asarray(comp_idx, dtype=np.int32)
    mu = np.asarray(mu, dtype=np.float32)
    rho = np.asarray(rho, dtype=np.float32)
    W1 = np.asarray(W1, dtype=np.float32)
    b1 = np.asarray(b1, dtype=np.float32)
    W2 = np.asarray(W2, dtype=np.float32)
    b2 = np.asarray(b2, dtype=np.float32)

    n = epsilon.shape[0]
    sigma = _softplus(rho)  # [C] float64

    sels = [np.nonzero(comp_idx == c)[0] for c in range(NB_COMP)]
    counts = [len(s) for s in sels]
    k_cap = max(128, -(-max(counts) // 128) * 128)

    nc = get_program(k_cap)

    in_maps = []
    for c in range(NB_COMP):
        sel = sels[c]
        x = np.zeros((128, KC1, k_cap), dtype=NP_BF16)
        if len(sel):
            xe = epsilon[sel].T.reshape(KC1, 128, len(sel)).transpose(1, 0, 2)
            x[:, :, : len(sel)] = xe.astype(NP_BF16)
        w1p = (W1[c].astype(np.float64) * sigma[c]).astype(np.float32)
        b1p = b1[c].astype(np.float64) + mu[c].astype(np.float64) @ W1[c].astype(
            np.float64
        )
        in_maps.append(
            {
                "x": x,
                "w1": np.ascontiguousarray(
                    w1p.reshape(KC1, 128, NB_NEUR).transpose(1, 0, 2)
                ).astype(NP_BF16),
                "b1": np.ascontiguousarray(
                    b1p.astype(np.float32).reshape(MC1, 128).T
                ),
                "w2": np.ascontiguousarray(
                    W2[c].reshape(KC2, 128, OUT_DIM).transpose(1, 0, 2)
                ).astype(NP_BF16),
                "b2": np.ascontiguousarray(b2[c].reshape(MC2, 128).T),
            }
        )

    res = run_bass_kernel_spmd(
        nc,
        in_maps,
        core_ids=list(range(N_CORES)),
        trace=_trace,
        trace_cores=list(range(N_CORES)) if _trace else None,
    )

    out = np.zeros((n, OUT_DIM), dtype=np.float32)
    for c in range(NB_COMP):
        sel = sels[c]
        if len(sel):
            y = np.asarray(res.results[c]["y"], dtype=np.float32)  # [128, MC2, k]
            out[sel] = y.transpose(2, 1, 0)[: len(sel)].reshape(len(sel), OUT_DIM)
    if _trace:
        return out, res
    return out


# revision 12
# speedup vs baseline: 1.0708x; 1.0708x over previous
"""GNN ensemble MoE-routing kernel for Trainium2 (8 NeuronCores).

Reference computes all 8 expert MLPs for every sample then selects one
(8x wasted FLOPs). This kernel routes on the host instead: samples are
gathered per expert, and core c runs ONLY expert c's MLP over the
samples routed to it (expert-parallel sharding).

Math folding (exact):
  lat = eps*sigma_c + mu_c  =>  lat @ W1_c = eps @ (sigma_c*W1_c) + mu_c@W1_c
so the device computes  sigmoid(eps @ W1p + b1p) @ W2 + b2  with
  W1p = sigma_c * W1_c,  b1p = b1_c + mu_c @ W1_c  (folded on host).

Device layout: features on SBUF partitions, samples on the free axis.
  epsT [512, K]  ->  H^T [1024, K]  ->  Y^T [512, K]
All matmul operands are bf16 (PSUM accumulation stays fp32; rel-err
~3e-3 vs the 2e-2 gate): halves HBM traffic and enables FWL weight
loads. The host pre-interleaves every tensor into a [128, blocks, n]
layout so each loads with ONE dma_start (a dma_start costs ~0.6us
serialized on the Sync sequencer; the fp32r baseline spent ~12us just
issuing its 54 descriptors, and late x-chunk issues starved the PE
mid-kernel). All x chunks are prefetched up front; outputs go back in
one coalesced DMA per chunk. First chunk is 256 wide so the PE starts
on a small DMA dependency; the trailing chunk is the k_cap remainder
so the end-of-kernel bias-add + store tail is short.
"""

from contextlib import ExitStack

import numpy as np

import concourse.bass as bass
import concourse.tile as tile
from concourse import bacc, mybir
from concourse.bass_utils import run_bass_kernel_spmd

NB_COMP = 8
LAT_DIM = 512
NB_NEUR = 1024
OUT_DIM = 512
N_CORES = 8
KC1, MC1 = LAT_DIM // 128, NB_NEUR // 128  # 4, 8
KC2, MC2 = NB_NEUR // 128, OUT_DIM // 128  # 8, 4

F32 = mybir.dt.float32
BF16 = mybir.dt.bfloat16
NP_BF16 = mybir.dt.np(BF16)
SIG = mybir.ActivationFunctionType.Sigmoid
IDENT = mybir.ActivationFunctionType.Identity

_program_cache = {}


def _make_chunks(k_cap):
    # First chunk 256 (small DMA dependency -> PE starts early), then
    # 512-wide chunks; the remainder is emitted last in <=256 pieces so
    # the end-of-kernel bias-add + store + DMA-receipt tail is short
    # (extra chunks are nearly free: ~3ns/matmul dispatch overhead warm).
    chunks = []
    rem = k_cap
    first = min(256, rem)
    chunks.append(first)
    rem -= first
    while rem >= 512:
        chunks.append(512)
        rem -= 512
    while rem > 0:
        ns = min(256, rem)
        if rem > 256:
            ns = rem - 128
        chunks.append(ns)
        rem -= ns
    out = []
    n0 = 0
    for ns in chunks:
        out.append((n0, ns))
        n0 += ns
    return out


def _build_program(k_cap):
    """One-expert MLP over k_cap samples; same program runs SPMD on all 8 cores."""
    chunks = _make_chunks(k_cap)

    nc = bacc.Bacc(
        "TRN2",
        target_bir_lowering=False,
        debug=False,
        enable_asserts=False,
        num_devices=N_CORES,
    )
    # Host-packed layouts, partition dim first everywhere:
    #   x[p, j, n]  = epsT[j*128 + p, n]       (j = mm1 contraction block)
    #   w1[p, j, m] = W1p[j*128 + p, m]
    #   w2[p, j, o] = W2[j*128 + p, o]
    #   b1[p, mc]   = b1p[mc*128 + p],  b2[p, oc] = b2[oc*128 + p]
    #   y[p, oc, n] = yT[oc*128 + p, n]
    x_d = nc.dram_tensor("x", [128, KC1, k_cap], BF16, kind="ExternalInput").ap()
    w1_d = nc.dram_tensor("w1", [128, KC1, NB_NEUR], BF16, kind="ExternalInput").ap()
    b1_d = nc.dram_tensor("b1", [128, MC1], F32, kind="ExternalInput").ap()
    w2_d = nc.dram_tensor("w2", [128, KC2, OUT_DIM], BF16, kind="ExternalInput").ap()
    b2_d = nc.dram_tensor("b2", [128, MC2], F32, kind="ExternalInput").ap()
    y_d = nc.dram_tensor("y", [128, MC2, k_cap], BF16, kind="ExternalOutput").ap()

    with tile.TileContext(nc) as tc, ExitStack() as ctx:
        wpool = ctx.enter_context(tc.tile_pool(name="weights", bufs=1))
        xpool = ctx.enter_context(tc.tile_pool(name="x", bufs=1))
        hpool = ctx.enter_context(tc.tile_pool(name="h", bufs=1))
        ypool = ctx.enter_context(tc.tile_pool(name="y", bufs=2))
        # One shared pool holding all 8 PSUM banks; mm1 keeps 8 accumulators
        # live (kc-outer order), mm2 4, cycling through the same slots.
        pspool = ctx.enter_context(tc.tile_pool(name="ps", bufs=8, space="PSUM"))

        # Warm the PE's HAM clock gate while the first DMAs are in flight:
        # the gate needs ~3.4us of sustained PE activity to lift the 1.2GHz
        # cold throttle, so a dozen dummy matmuls on a memset tile make the
        # real matmuls run at 2.4GHz from their first instruction.
        warm = wpool.tile([128, 512], BF16, tag="warm")
        nc.gpsimd.memset(warm[:], 0.0)
        wps = pspool.tile([128, 512], F32, tag="ps", name="warm_ps")
        for _ in range(26):
            nc.tensor.matmul(wps[:], warm[:, :128], warm[:], start=True, stop=True)

        # Startup is DMA-supply-bound: the sync HWDGE ring delivers
        # ~275-320 GB/s with ~1.5us start latency and ~1.5us
        # completion-receipt lag per dma_start (the ACT ring measured 2.3x
        # slower per byte - don't use it). The warmup matmul count above is
        # sized so the real matmul stream starts right as this FIFO can
        # sustain it; blocks are ordered by when the PE needs them.
        n0_0, ns_0 = chunks[0]
        w1a = wpool.tile([128, NB_NEUR], BF16, tag="w1a")
        nc.sync.dma_start(w1a[:], w1_d[:, 0, :])
        xt = []
        t = xpool.tile([128, KC1, ns_0], BF16, tag="x0", name="x_0")
        nc.sync.dma_start(t[:], x_d[:, :, n0_0 : n0_0 + ns_0])
        xt.append(t)
        w1b = wpool.tile([128, NB_NEUR], BF16, tag="w1b")
        nc.sync.dma_start(w1b[:], w1_d[:, 1, :])
        b1t = wpool.tile([128, MC1], F32, tag="b1")
        nc.sync.dma_start(b1t[:], b1_d[:])
        w1c = wpool.tile([128, KC1 - 2, NB_NEUR], BF16, tag="w1c")
        nc.sync.dma_start(w1c[:], w1_d[:, 2:, :])
        w2t = wpool.tile([128, KC2, OUT_DIM], BF16, tag="w2")
        nc.sync.dma_start(w2t[:], w2_d[:, :, :])
        b2t = wpool.tile([128, MC2], F32, tag="b2")
        nc.sync.dma_start(b2t[:], b2_d[:])
        if len(chunks) > 1:
            n0, ns = chunks[1]
            t = xpool.tile([128, KC1, ns], BF16, tag="x1", name="x_1")
            nc.sync.dma_start(t[:], x_d[:, :, n0 : n0 + ns])
            xt.append(t)
        if len(chunks) > 2:
            n0r = chunks[2][0]
            nsr = k_cap - n0r
            xr = xpool.tile([128, KC1, nsr], BF16, tag="xr", name="x_rest")
            nc.sync.dma_start(xr[:], x_d[:, :, n0r:])
        else:
            xr = None

        def x_ap(ci, kc):
            n0, ns = chunks[ci]
            if ci < 2:
                return xt[ci][:, kc, :]
            return xr[:, kc, n0 - n0r : n0 - n0r + ns]

        def w1_ap(kc, mc):
            if kc == 0:
                return w1a[:, mc * 128 : (mc + 1) * 128]
            if kc == 1:
                return w1b[:, mc * 128 : (mc + 1) * 128]
            return w1c[:, kc - 2, mc * 128 : (mc + 1) * 128]

        for ci, (n0, ns) in enumerate(chunks):
            ht = []
            ps1 = [
                pspool.tile([128, ns], F32, tag="ps", name=f"ps1_{ci}_{i}")
                for i in range(MC1)
            ]
            for kc in range(KC1):
                for mc in range(MC1):
                    nc.tensor.matmul(
                        ps1[mc][:],
                        w1_ap(kc, mc),
                        x_ap(ci, kc),
                        start=(kc == 0),
                        stop=(kc == KC1 - 1),
                    )
                    if kc == KC1 - 1:
                        h = hpool.tile([128, ns], BF16, tag=f"h{mc}")
                        nc.scalar.activation(
                            h[:], ps1[mc][:], SIG, bias=b1t[:, mc : mc + 1]
                        )
                        ht.append(h)

            yt = ypool.tile([128, MC2, ns], BF16, tag="y", name=f"y_{ci}")
            ps2 = [
                pspool.tile([128, ns], F32, tag="ps", name=f"ps2_{ci}_{i}")
                for i in range(MC2)
            ]
            for kc in range(KC2):
                for oc in range(MC2):
                    nc.tensor.matmul(
                        ps2[oc][:],
                        w2t[:, kc, oc * 128 : (oc + 1) * 128],
                        ht[kc][:],
                        start=(kc == 0),
                        stop=(kc == KC2 - 1),
                    )
                    if kc == KC2 - 1:
                        # Alternate DVE / ACT so the 4 bias-adds of the last
                        # chunk drain on two engines instead of one.
                        if oc % 2 == 0:
                            nc.vector.tensor_scalar_add(
                                yt[:, oc, :], ps2[oc][:], b2t[:, oc : oc + 1]
                            )
                        else:
                            nc.scalar.activation(
                                yt[:, oc, :],
                                ps2[oc][:],
                                IDENT,
                                bias=b2t[:, oc : oc + 1],
                            )
            nc.sync.dma_start(y_d[:, :, n0 : n0 + ns], yt[:])

    nc.compile()
    return nc


def get_program(k_cap):
    if k_cap not in _program_cache:
        _program_cache[k_cap] = _build_program(k_cap)
    return _program_cache[k_cap]


def _softplus(x):
    x = x.astype(np.float64)
    return np.maximum(x, 0.0) + np.log1p(np.exp(-np.abs(x)))


def kernel(epsilon, comp_idx, mu, rho, W1, b1, W2, b2, _trace=False):
    epsilon = np.asarray(epsilon, dtype=np.float32)
    comp_idx = np.asarray(comp_idx, dtype=np.int32)
    mu = np.asarray(mu, dtype=np.float32)
    rho = np.asarray(rho, dtype=np.float32)
    W1 = np.asarray(W1, dtype=np.float32)
    b1 = np.asarray(b1, dtype=np.float32)
    W2 = np.asarray(W2, dtype=np.float32)
    b2 = np.asarray(b2, dtype=np.float32)

    n = epsilon.shape[0]
    sigma = _softplus(rho)  # [C] float64

    sels = [np.nonzero(comp_idx == c)[0] for c in range(NB_COMP)]
    counts = [len(s) for s in sels]
    k_cap = max(128, -(-max(counts) // 8) * 8)

    nc = get_program(k_cap)

    in_maps = []
    for c in range(NB_COMP):
        sel = sels[c]
        x = np.zeros((128, KC1, k_cap), dtype=NP_BF16)
        if len(sel):
            xe = epsilon[sel].T.reshape(KC1, 128, len(sel)).transpose(1, 0, 2)
            x[:, :, : len(sel)] = xe.astype(NP_BF16)
        w1p = (W1[c].astype(np.float64) * sigma[c]).astype(np.float32)
        b1p = b1[c].astype(np.float64) + mu[c].astype(np.float64) @ W1[c].astype(
            np.float64
        )
        in_maps.append(
            {
                "x": x,
                "w1": np.ascontiguousarray(
                    w1p.reshape(KC1, 128, NB_NEUR).transpose(1, 0, 2)
                ).astype(NP_BF16),
                "b1": np.ascontiguousarray(
                    b1p.astype(np.float32).reshape(MC1, 128).T
                ),
                "w2": np.ascontiguousarray(
                    W2[c].reshape(KC2, 128, OUT_DIM).transpose(1, 0, 2)
                ).astype(NP_BF16),
                "b2": np.ascontiguousarray(b2[c].reshape(MC2, 128).T),
            }
        )

    res = run_bass_kernel_spmd(
        nc,
        in_maps,
        core_ids=list(range(N_CORES)),
        trace=_trace,
        trace_cores=list(range(N_CORES)) if _trace else None,
    )

    out = np.zeros((n, OUT_DIM), dtype=np.float32)
    for c in range(NB_COMP):
        sel = sels[c]
        if len(sel):
            y = np.asarray(res.results[c]["y"], dtype=np.float32)  # [128, MC2, k]
            out[sel] = y.transpose(2, 1, 0)[: len(sel)].reshape(len(sel), OUT_DIM)
    if _trace:
        return out, res
    return out


# revision 17
# speedup vs baseline: 1.1020x; 1.0291x over previous
"""GNN ensemble MoE-routing kernel for Trainium2 (8 NeuronCores).

Reference computes all 8 expert MLPs for every sample then selects one
(8x wasted FLOPs). This kernel routes on the host instead: samples are
gathered per expert, and core c runs ONLY expert c's MLP over the
samples routed to it (expert-parallel sharding).

Math folding (exact):
  lat = eps*sigma_c + mu_c  =>  lat @ W1_c = eps @ (sigma_c*W1_c) + mu_c@W1_c
so the device computes  sigmoid(eps @ W1p + b1p) @ W2 + b2  with
  W1p = sigma_c * W1_c,  b1p = b1_c + mu_c @ W1_c  (folded on host).

Device layout: features on SBUF partitions, samples on the free axis.
  epsT [512, K]  ->  H^T [1024, K]  ->  Y^T [512, K]
All matmul operands are bf16 (PSUM accumulation stays fp32; rel-err
~3e-3 vs the 2e-2 gate): halves HBM traffic and enables FWL weight
loads. The host pre-interleaves every tensor into a [128, blocks, n]
layout so each loads with ONE dma_start (a dma_start costs ~0.6us
serialized on the Sync sequencer; the fp32r baseline spent ~12us just
issuing its 54 descriptors, and late x-chunk issues starved the PE
mid-kernel). All x chunks are prefetched up front; outputs go back in
one coalesced DMA per chunk. First chunk is 256 wide so the PE starts
on a small DMA dependency; the trailing chunk is the k_cap remainder
so the end-of-kernel bias-add + store tail is short.
"""

from contextlib import ExitStack

import numpy as np

import concourse.bass as bass
import concourse.tile as tile
from concourse import bacc, mybir
from concourse.bass_utils import run_bass_kernel_spmd

NB_COMP = 8
LAT_DIM = 512
NB_NEUR = 1024
OUT_DIM = 512
N_CORES = 8
KC1, MC1 = LAT_DIM // 128, NB_NEUR // 128  # 4, 8
KC2, MC2 = NB_NEUR // 128, OUT_DIM // 128  # 8, 4

F32 = mybir.dt.float32
BF16 = mybir.dt.bfloat16
NP_BF16 = mybir.dt.np(BF16)
SIG = mybir.ActivationFunctionType.Sigmoid
IDENT = mybir.ActivationFunctionType.Identity

_program_cache = {}


def _make_chunks(k_cap):
    # 512-wide chunks (PSUM-bank limit), remainder split so the final
    # chunk is 128 wide: the end-of-kernel bias-add + store + DMA-receipt
    # tail scales with the last chunk, while extra chunks cost almost
    # nothing (~8ns/matmul dispatch overhead warm).
    chunks = []
    rem = k_cap
    while rem >= 512 + 128:
        chunks.append(512)
        rem -= 512
    if rem > 256:
        chunks.append(rem - 128)
        chunks.append(128)
    elif rem:
        chunks.append(rem)
    out = []
    n0 = 0
    for ns in chunks:
        out.append((n0, ns))
        n0 += ns
    return out


def _build_program(k_cap):
    """One-expert MLP over k_cap samples; same program runs SPMD on all 8 cores."""
    chunks = _make_chunks(k_cap)

    nc = bacc.Bacc(
        "TRN2",
        target_bir_lowering=False,
        debug=False,
        enable_asserts=False,
        num_devices=N_CORES,
    )
    # Host-packed layouts, partition dim first everywhere:
    #   x[p, j, n]  = epsT[j*128 + p, n]       (j = mm1 contraction block)
    #   w1[p, j, m] = W1p[j*128 + p, m]
    #   w2[p, j, o] = W2[j*128 + p, o]
    #   b1[p, mc]   = b1p[mc*128 + p],  b2[p, oc] = b2[oc*128 + p]
    #   y[p, oc, n] = yT[oc*128 + p, n]
    x_d = nc.dram_tensor("x", [128, KC1, k_cap], BF16, kind="ExternalInput").ap()
    w1_d = nc.dram_tensor("w1", [128, KC1, NB_NEUR], BF16, kind="ExternalInput").ap()
    b1_d = nc.dram_tensor("b1", [128, MC1], F32, kind="ExternalInput").ap()
    w2_d = nc.dram_tensor("w2", [128, KC2, OUT_DIM], BF16, kind="ExternalInput").ap()
    b2_d = nc.dram_tensor("b2", [128, MC2], F32, kind="ExternalInput").ap()
    y_d = nc.dram_tensor("y", [128, MC2, k_cap], BF16, kind="ExternalOutput").ap()

    with tile.TileContext(nc) as tc, ExitStack() as ctx:
        wpool = ctx.enter_context(tc.tile_pool(name="weights", bufs=1))
        xpool = ctx.enter_context(tc.tile_pool(name="x", bufs=1))
        hpool = ctx.enter_context(tc.tile_pool(name="h", bufs=1))
        ypool = ctx.enter_context(tc.tile_pool(name="y", bufs=2))
        # One shared pool holding all 8 PSUM banks; mm1 keeps 8 accumulators
        # live (kc-outer order), mm2 4, cycling through the same slots.
        pspool = ctx.enter_context(tc.tile_pool(name="ps", bufs=8, space="PSUM"))

        # Warm the PE's HAM clock gate while the first DMAs are in flight:
        # the gate needs ~3.4us of sustained PE activity to lift the 1.2GHz
        # cold throttle, so a dozen dummy matmuls on a memset tile make the
        # real matmuls run at 2.4GHz from their first instruction.
        warm = wpool.tile([128, 512], BF16, tag="warm")
        nc.gpsimd.memset(warm[:], 0.0)
        wps = pspool.tile([128, 512], F32, tag="ps", name="warm_ps")
        for _ in range(20):
            nc.tensor.matmul(wps[:], warm[:, :128], warm[:], start=True, stop=True)

        # Startup is DMA-supply-bound: the sync HWDGE ring delivers
        # ~275-320 GB/s with ~1.5us start latency and ~1.5us
        # completion-receipt lag per dma_start (the ACT ring measured 2.3x
        # slower per byte - don't use it). The warmup matmul count above is
        # sized so the real matmul stream starts right as this FIFO can
        # sustain it; blocks are ordered by when the PE needs them.
        n0_0, ns_0 = chunks[0]
        w1a = wpool.tile([128, NB_NEUR], BF16, tag="w1a")
        nc.sync.dma_start(w1a[:], w1_d[:, 0, :])
        xt = []
        t = xpool.tile([128, KC1, ns_0], BF16, tag="x0", name="x_0")
        nc.sync.dma_start(t[:], x_d[:, :, n0_0 : n0_0 + ns_0])
        xt.append(t)
        w1b = wpool.tile([128, NB_NEUR], BF16, tag="w1b")
        nc.sync.dma_start(w1b[:], w1_d[:, 1, :])
        b1t = wpool.tile([128, MC1], F32, tag="b1")
        nc.sync.dma_start(b1t[:], b1_d[:])
        w1c = wpool.tile([128, KC1 - 2, NB_NEUR], BF16, tag="w1c")
        nc.sync.dma_start(w1c[:], w1_d[:, 2:, :])
        w2a = wpool.tile([128, 2, OUT_DIM], BF16, tag="w2a")
        nc.sync.dma_start(w2a[:], w2_d[:, :2, :])
        w2b = wpool.tile([128, KC2 - 2, OUT_DIM], BF16, tag="w2b")
        nc.sync.dma_start(w2b[:], w2_d[:, 2:, :])
        b2t = wpool.tile([128, MC2], F32, tag="b2")
        nc.sync.dma_start(b2t[:], b2_d[:])

        def w2_ap(kc, oc):
            if kc < 2:
                return w2a[:, kc, oc * 128 : (oc + 1) * 128]
            return w2b[:, kc - 2, oc * 128 : (oc + 1) * 128]
        if len(chunks) > 1:
            n0, ns = chunks[1]
            t = xpool.tile([128, KC1, ns], BF16, tag="x1", name="x_1")
            nc.sync.dma_start(t[:], x_d[:, :, n0 : n0 + ns])
            xt.append(t)
        if len(chunks) > 2:
            n0r = chunks[2][0]
            nsr = k_cap - n0r
            xr = xpool.tile([128, KC1, nsr], BF16, tag="xr", name="x_rest")
            nc.sync.dma_start(xr[:], x_d[:, :, n0r:])
        else:
            xr = None

        def x_ap(ci, kc):
            n0, ns = chunks[ci]
            if ci < 2:
                return xt[ci][:, kc, :]
            return xr[:, kc, n0 - n0r : n0 - n0r + ns]

        def w1_ap(kc, mc):
            if kc == 0:
                return w1a[:, mc * 128 : (mc + 1) * 128]
            if kc == 1:
                return w1b[:, mc * 128 : (mc + 1) * 128]
            return w1c[:, kc - 2, mc * 128 : (mc + 1) * 128]

        for ci, (n0, ns) in enumerate(chunks):
            ht = []
            ps1 = [
                pspool.tile([128, ns], F32, tag="ps", name=f"ps1_{ci}_{i}")
                for i in range(MC1)
            ]
            for kc in range(KC1):
                for mc in range(MC1):
                    nc.tensor.matmul(
                        ps1[mc][:],
                        w1_ap(kc, mc),
                        x_ap(ci, kc),
                        start=(kc == 0),
                        stop=(kc == KC1 - 1),
                    )
                    if kc == KC1 - 1:
                        h = hpool.tile([128, ns], BF16, tag=f"h{mc}")
                        nc.scalar.activation(
                            h[:], ps1[mc][:], SIG, bias=b1t[:, mc : mc + 1]
                        )
                        ht.append(h)

            yt = ypool.tile([128, MC2, ns], BF16, tag="y", name=f"y_{ci}")
            ps2 = [
                pspool.tile([128, ns], F32, tag="ps", name=f"ps2_{ci}_{i}")
                for i in range(MC2)
            ]
            for kc in range(KC2):
                for oc in range(MC2):
                    nc.tensor.matmul(
                        ps2[oc][:],
                        w2_ap(kc, oc),
                        ht[kc][:],
                        start=(kc == 0),
                        stop=(kc == KC2 - 1),
                    )
                    if kc == KC2 - 1:
                        # Alternate DVE / ACT so the 4 bias-adds of the last
                        # chunk drain on two engines instead of one.
                        if oc % 2 == 0:
                            nc.vector.tensor_scalar_add(
                                yt[:, oc, :], ps2[oc][:], b2t[:, oc : oc + 1]
                            )
                        else:
                            nc.scalar.activation(
                                yt[:, oc, :],
                                ps2[oc][:],
                                IDENT,
                                bias=b2t[:, oc : oc + 1],
                            )
            nc.sync.dma_start(y_d[:, :, n0 : n0 + ns], yt[:])

    nc.compile()
    return nc


def get_program(k_cap):
    if k_cap not in _program_cache:
        _program_cache[k_cap] = _build_program(k_cap)
    return _program_cache[k_cap]


def _softplus(x):
    x = x.astype(np.float64)
    return np.maximum(x, 0.0) + np.log1p(np.exp(-np.abs(x)))


def kernel(epsilon, comp_idx, mu, rho, W1, b1, W2, b2, _trace=False):
    epsilon = np.asarray(epsilon, dtype=np.float32)
    comp_idx = np.asarray(comp_idx, dtype=np.int32)
    mu = np.asarray(mu, dtype=np.float32)
    rho = np.asarray(rho, dtype=np.float32)
    W1 = np.asarray(W1, dtype=np.float32)
    b1 = np.asarray(b1, dtype=np.float32)
    W2 = np.asarray(W2, dtype=np.float32)
    b2 = np.asarray(b2, dtype=np.float32)

    n = epsilon.shape[0]
    sigma = _softplus(rho)  # [C] float64

    sels = [np.nonzero(comp_idx == c)[0] for c in range(NB_COMP)]
    counts = [len(s) for s in sels]
    k_cap = max(128, -(-max(counts) // 8) * 8)

    nc = get_program(k_cap)

    in_maps = []
    for c in range(NB_COMP):
        sel = sels[c]
        x = np.zeros((128, KC1, k_cap), dtype=NP_BF16)
        if len(sel):
            xe = epsilon[sel].T.reshape(KC1, 128, len(sel)).transpose(1, 0, 2)
            x[:, :, : len(sel)] = xe.astype(NP_BF16)
        w1p = (W1[c].astype(np.float64) * sigma[c]).astype(np.float32)
        b1p = b1[c].astype(np.float64) + mu[c].astype(np.float64) @ W1[c].astype(
            np.float64
        )
        in_maps.append(
            {
                "x": x,
                "w1": np.ascontiguousarray(
                    w1p.reshape(KC1, 128, NB_NEUR).transpose(1, 0, 2)
                ).astype(NP_BF16),
                "b1": np.ascontiguousarray(
                    b1p.astype(np.float32).reshape(MC1, 128).T
                ),
                "w2": np.ascontiguousarray(
                    W2[c].reshape(KC2, 128, OUT_DIM).transpose(1, 0, 2)
                ).astype(NP_BF16),
                "b2": np.ascontiguousarray(b2[c].reshape(MC2, 128).T),
            }
        )

    res = run_bass_kernel_spmd(
        nc,
        in_maps,
        core_ids=list(range(N_CORES)),
        trace=_trace,
        trace_cores=list(range(N_CORES)) if _trace else None,
    )

    out = np.zeros((n, OUT_DIM), dtype=np.float32)
    for c in range(NB_COMP):
        sel = sels[c]
        if len(sel):
            y = np.asarray(res.results[c]["y"], dtype=np.float32)  # [128, MC2, k]
            out[sel] = y.transpose(2, 1, 0)[: len(sel)].reshape(len(sel), OUT_DIM)
    if _trace:
        return out, res
    return out


# revision 19
# speedup vs baseline: 1.1038x; 1.0017x over previous
"""GNN ensemble MoE-routing kernel for Trainium2 (8 NeuronCores).

Reference computes all 8 expert MLPs for every sample then selects one
(8x wasted FLOPs). This kernel routes on the host instead: samples are
gathered per expert, and core c runs ONLY expert c's MLP over the
samples routed to it (expert-parallel sharding).

Math folding (exact):
  lat = eps*sigma_c + mu_c  =>  lat @ W1_c = eps @ (sigma_c*W1_c) + mu_c@W1_c
so the device computes  sigmoid(eps @ W1p + b1p) @ W2 + b2  with
  W1p = sigma_c * W1_c,  b1p = b1_c + mu_c @ W1_c  (folded on host).

Device layout: features on SBUF partitions, samples on the free axis.
  epsT [512, K]  ->  H^T [1024, K]  ->  Y^T [512, K]
All matmul operands are bf16 (PSUM accumulation stays fp32; rel-err
~3e-3 vs the 2e-2 gate): 1 PE cycle/column like fp32r but half the HBM
traffic. The host pre-interleaves every tensor into a [128, blocks, n]
layout so each loads in a couple of coalesced dma_starts (a dma_start
costs ~0.6us serialized on the Sync sequencer; the fp32r baseline
spent ~12us just issuing its 54 descriptors, and late x-chunk issues
starved the PE mid-kernel). Startup is supply-bound on the sync HWDGE
ring (~300 GB/s + ~1.5us start + ~1.5us completion-receipt latency),
so dummy warm-up matmuls both lift the PE HAM clock gate (1.2 -> 2.4
GHz after ~3.4us of sustained activity) and absorb the supply latency;
input blocks are FIFO-ordered to land exactly when the matmul stream
reaches them (measured: zero PE gaps). k_cap is the max per-expert
count rounded to 8 (the moving dim needs no 128 alignment), and the
final 128-wide chunk keeps the bias-add + store + DMA-receipt tail
short. Both graded-span endpoints are fixed overhead: ~0.5us of
framework const-pool memsets up front and a ~6.5us NEFF epilogue that
clears all 254 semaphores at ~150ns each.
"""

from contextlib import ExitStack

import numpy as np

import concourse.tile as tile
from concourse import bacc, mybir
from concourse.bass_utils import run_bass_kernel_spmd

NB_COMP = 8
LAT_DIM = 512
NB_NEUR = 1024
OUT_DIM = 512
N_CORES = 8
KC1, MC1 = LAT_DIM // 128, NB_NEUR // 128  # 4, 8
KC2, MC2 = NB_NEUR // 128, OUT_DIM // 128  # 8, 4

F32 = mybir.dt.float32
BF16 = mybir.dt.bfloat16
NP_BF16 = mybir.dt.np(BF16)
SIG = mybir.ActivationFunctionType.Sigmoid
IDENT = mybir.ActivationFunctionType.Identity

_program_cache = {}


def _make_chunks(k_cap):
    # 512-wide chunks (PSUM-bank limit), remainder split so the final
    # chunk is 128 wide: the end-of-kernel bias-add + store + DMA-receipt
    # tail scales with the last chunk, while extra chunks cost almost
    # nothing (~8ns/matmul dispatch overhead warm).
    chunks = []
    rem = k_cap
    while rem >= 512 + 128:
        chunks.append(512)
        rem -= 512
    if rem > 256:
        chunks.append(rem - 128)
        chunks.append(128)
    elif rem:
        chunks.append(rem)
    out = []
    n0 = 0
    for ns in chunks:
        out.append((n0, ns))
        n0 += ns
    return out


def _build_program(k_cap):
    """One-expert MLP over k_cap samples; same program runs SPMD on all 8 cores."""
    chunks = _make_chunks(k_cap)

    nc = bacc.Bacc(
        "TRN2",
        target_bir_lowering=False,
        debug=False,
        enable_asserts=False,
        num_devices=N_CORES,
    )
    # Host-packed layouts, partition dim first everywhere:
    #   x[p, j, n]  = epsT[j*128 + p, n]       (j = mm1 contraction block)
    #   w1[p, j, m] = W1p[j*128 + p, m]
    #   w2[p, j, o] = W2[j*128 + p, o]
    #   b1[p, mc]   = b1p[mc*128 + p],  b2[p, oc] = b2[oc*128 + p]
    #   y[p, oc, n] = yT[oc*128 + p, n]
    x_d = nc.dram_tensor("x", [128, KC1, k_cap], BF16, kind="ExternalInput").ap()
    w1_d = nc.dram_tensor("w1", [128, KC1, NB_NEUR], BF16, kind="ExternalInput").ap()
    b1_d = nc.dram_tensor("b1", [128, MC1], F32, kind="ExternalInput").ap()
    w2_d = nc.dram_tensor("w2", [128, KC2, OUT_DIM], BF16, kind="ExternalInput").ap()
    b2_d = nc.dram_tensor("b2", [128, MC2], F32, kind="ExternalInput").ap()
    y_d = nc.dram_tensor("y", [128, MC2, k_cap], BF16, kind="ExternalOutput").ap()

    with tile.TileContext(nc) as tc, ExitStack() as ctx:
        wpool = ctx.enter_context(tc.tile_pool(name="weights", bufs=1))
        xpool = ctx.enter_context(tc.tile_pool(name="x", bufs=1))
        hpool = ctx.enter_context(tc.tile_pool(name="h", bufs=1))
        ypool = ctx.enter_context(tc.tile_pool(name="y", bufs=2))
        # One shared pool holding all 8 PSUM banks; mm1 keeps 8 accumulators
        # live (kc-outer order), mm2 4, cycling through the same slots.
        pspool = ctx.enter_context(tc.tile_pool(name="ps", bufs=8, space="PSUM"))

        # Warm the PE's HAM clock gate while the first DMAs are in flight:
        # the gate needs ~3.4us of sustained PE activity to lift the 1.2GHz
        # cold throttle, so a dozen dummy matmuls on a memset tile make the
        # real matmuls run at 2.4GHz from their first instruction.
        warm = wpool.tile([128, 512], BF16, tag="warm")
        nc.gpsimd.memset(warm[:], 0.0)
        wps = pspool.tile([128, 512], F32, tag="ps", name="warm_ps")
        for _ in range(17):
            nc.tensor.matmul(wps[:], warm[:, :128], warm[:], start=True, stop=True)

        # Startup is DMA-supply-bound: the sync HWDGE ring delivers
        # ~275-320 GB/s with ~1.5us start latency and ~1.5us
        # completion-receipt lag per dma_start (the ACT ring measured 2.3x
        # slower per byte - don't use it). The warmup matmul count above is
        # sized so the real matmul stream starts right as this FIFO can
        # sustain it; blocks are ordered by when the PE needs them.
        n0_0, ns_0 = chunks[0]
        w1a = wpool.tile([128, NB_NEUR], BF16, tag="w1a")
        nc.sync.dma_start(w1a[:], w1_d[:, 0, :])
        xt = []
        t = xpool.tile([128, KC1, ns_0], BF16, tag="x0", name="x_0")
        nc.sync.dma_start(t[:], x_d[:, :, n0_0 : n0_0 + ns_0])
        xt.append(t)
        w1b = wpool.tile([128, NB_NEUR], BF16, tag="w1b")
        nc.sync.dma_start(w1b[:], w1_d[:, 1, :])
        b1t = wpool.tile([128, MC1], F32, tag="b1")
        nc.sync.dma_start(b1t[:], b1_d[:])
        w1c = wpool.tile([128, KC1 - 2, NB_NEUR], BF16, tag="w1c")
        nc.sync.dma_start(w1c[:], w1_d[:, 2:, :])
        w2a = wpool.tile([128, 2, OUT_DIM], BF16, tag="w2a")
        nc.sync.dma_start(w2a[:], w2_d[:, :2, :])
        w2b = wpool.tile([128, KC2 - 2, OUT_DIM], BF16, tag="w2b")
        nc.sync.dma_start(w2b[:], w2_d[:, 2:, :])
        b2t = wpool.tile([128, MC2], F32, tag="b2")
        nc.sync.dma_start(b2t[:], b2_d[:])

        def w2_ap(kc, oc):
            if kc < 2:
                return w2a[:, kc, oc * 128 : (oc + 1) * 128]
            return w2b[:, kc - 2, oc * 128 : (oc + 1) * 128]
        if len(chunks) > 1:
            n0, ns = chunks[1]
            t = xpool.tile([128, KC1, ns], BF16, tag="x1", name="x_1")
            nc.sync.dma_start(t[:], x_d[:, :, n0 : n0 + ns])
            xt.append(t)
        if len(chunks) > 2:
            n0r = chunks[2][0]
            nsr = k_cap - n0r
            xr = xpool.tile([128, KC1, nsr], BF16, tag="xr", name="x_rest")
            nc.sync.dma_start(xr[:], x_d[:, :, n0r:])
        else:
            xr = None

        def x_ap(ci, kc):
            n0, ns = chunks[ci]
            if ci < 2:
                return xt[ci][:, kc, :]
            return xr[:, kc, n0 - n0r : n0 - n0r + ns]

        def w1_ap(kc, mc):
            if kc == 0:
                return w1a[:, mc * 128 : (mc + 1) * 128]
            if kc == 1:
                return w1b[:, mc * 128 : (mc + 1) * 128]
            return w1c[:, kc - 2, mc * 128 : (mc + 1) * 128]

        for ci, (n0, ns) in enumerate(chunks):
            ht = []
            ps1 = [
                pspool.tile([128, ns], F32, tag="ps", name=f"ps1_{ci}_{i}")
                for i in range(MC1)
            ]
            for kc in range(KC1):
                for mc in range(MC1):
                    nc.tensor.matmul(
                        ps1[mc][:],
                        w1_ap(kc, mc),
                        x_ap(ci, kc),
                        start=(kc == 0),
                        stop=(kc == KC1 - 1),
                    )
                    if kc == KC1 - 1:
                        h = hpool.tile([128, ns], BF16, tag=f"h{mc}")
                        nc.scalar.activation(
                            h[:], ps1[mc][:], SIG, bias=b1t[:, mc : mc + 1]
                        )
                        ht.append(h)

            yt = ypool.tile([128, MC2, ns], BF16, tag="y", name=f"y_{ci}")
            ps2 = [
                pspool.tile([128, ns], F32, tag="ps", name=f"ps2_{ci}_{i}")
                for i in range(MC2)
            ]
            for kc in range(KC2):
                for oc in range(MC2):
                    nc.tensor.matmul(
                        ps2[oc][:],
                        w2_ap(kc, oc),
                        ht[kc][:],
                        start=(kc == 0),
                        stop=(kc == KC2 - 1),
                    )
                    if kc == KC2 - 1:
                        # Alternate DVE / ACT so the 4 bias-adds of the last
                        # chunk drain on two engines instead of one.
                        if oc % 2 == 0:
                            nc.vector.tensor_scalar_add(
                                yt[:, oc, :], ps2[oc][:], b2t[:, oc : oc + 1]
                            )
                        else:
                            nc.scalar.activation(
                                yt[:, oc, :],
                                ps2[oc][:],
                                IDENT,
                                bias=b2t[:, oc : oc + 1],
                            )
            nc.sync.dma_start(y_d[:, :, n0 : n0 + ns], yt[:])

    nc.compile()
    return nc


def get_program(k_cap):
    if k_cap not in _program_cache:
        _program_cache[k_cap] = _build_program(k_cap)
    return _program_cache[k_cap]


def _softplus(x):
    x = x.astype(np.float64)
    return np.maximum(x, 0.0) + np.log1p(np.exp(-np.abs(x)))


def kernel(epsilon, comp_idx, mu, rho, W1, b1, W2, b2, _trace=False):
    epsilon = np.asarray(epsilon, dtype=np.float32)
    comp_idx = np.asarray(comp_idx, dtype=np.int32)
    mu = np.asarray(mu, dtype=np.float32)
    rho = np.asarray(rho, dtype=np.float32)
    W1 = np.asarray(W1, dtype=np.float32)
    b1 = np.asarray(b1, dtype=np.float32)
    W2 = np.asarray(W2, dtype=np.float32)
    b2 = np.asarray(b2, dtype=np.float32)

    n = epsilon.shape[0]
    sigma = _softplus(rho)  # [C] float64

    sels = [np.nonzero(comp_idx == c)[0] for c in range(NB_COMP)]
    counts = [len(s) for s in sels]
    k_cap = max(128, -(-max(counts) // 8) * 8)

    nc = get_program(k_cap)

    in_maps = []
    for c in range(NB_COMP):
        sel = sels[c]
        x = np.zeros((128, KC1, k_cap), dtype=NP_BF16)
        if len(sel):
            xe = epsilon[sel].T.reshape(KC1, 128, len(sel)).transpose(1, 0, 2)
            x[:, :, : len(sel)] = xe.astype(NP_BF16)
        w1p = (W1[c].astype(np.float64) * sigma[c]).astype(np.float32)
        b1p = b1[c].astype(np.float64) + mu[c].astype(np.float64) @ W1[c].astype(
            np.float64
        )
        in_maps.append(
            {
                "x": x,
                "w1": np.ascontiguousarray(
                    w1p.reshape(KC1, 128, NB_NEUR).transpose(1, 0, 2)
                ).astype(NP_BF16),
                "b1": np.ascontiguousarray(
                    b1p.astype(np.float32).reshape(MC1, 128).T
                ),
                "w2": np.ascontiguousarray(
                    W2[c].reshape(KC2, 128, OUT_DIM).transpose(1, 0, 2)
                ).astype(NP_BF16),
                "b2": np.ascontiguousarray(b2[c].reshape(MC2, 128).T),
            }
        )

    res = run_bass_kernel_spmd(
        nc,
        in_maps,
        core_ids=list(range(N_CORES)),
        trace=_trace,
        trace_cores=list(range(N_CORES)) if _trace else None,
    )

    out = np.zeros((n, OUT_DIM), dtype=np.float32)
    for c in range(NB_COMP):
        sel = sels[c]
        if len(sel):
            y = np.asarray(res.results[c]["y"], dtype=np.float32)  # [128, MC2, k]
            out[sel] = y.transpose(2, 1, 0)[: len(sel)].reshape(len(sel), OUT_DIM)
    if _trace:
        return out, res
    return out


# revision 20
# speedup vs baseline: 1.1138x; 1.0090x over previous
"""GNN ensemble MoE-routing kernel for Trainium2 (8 NeuronCores).

Reference computes all 8 expert MLPs for every sample then selects one
(8x wasted FLOPs). This kernel routes on the host instead: samples are
gathered per expert, and core c runs ONLY expert c's MLP over the
samples routed to it (expert-parallel sharding).

Math folding (exact):
  lat = eps*sigma_c + mu_c  =>  lat @ W1_c = eps @ (sigma_c*W1_c) + mu_c@W1_c
so the device computes  sigmoid(eps @ W1p + b1p) @ W2 + b2  with
  W1p = sigma_c * W1_c,  b1p = b1_c + mu_c @ W1_c  (folded on host).

Device layout: features on SBUF partitions, samples on the free axis.
  epsT [512, K]  ->  H^T [1024, K]  ->  Y^T [512, K]
All matmul operands are bf16 (PSUM accumulation stays fp32; rel-err
~3e-3 vs the 2e-2 gate): 1 PE cycle/column like fp32r but half the HBM
traffic. The host pre-interleaves every tensor into a [128, blocks, n]
layout so each loads in a couple of coalesced dma_starts (a dma_start
costs ~0.6us serialized on the Sync sequencer; the fp32r baseline
spent ~12us just issuing its 54 descriptors, and late x-chunk issues
starved the PE mid-kernel). Startup is supply-bound on the sync HWDGE
ring (~300 GB/s + ~1.5us start + ~1.5us completion-receipt latency),
so dummy warm-up matmuls both lift the PE HAM clock gate (1.2 -> 2.4
GHz after ~3.4us of sustained activity) and absorb the supply latency;
input blocks are FIFO-ordered to land exactly when the matmul stream
reaches them (measured: zero PE gaps). k_cap is the max per-expert
count rounded to 8 (the moving dim needs no 128 alignment), and the
final 128-wide chunk keeps the bias-add + store + DMA-receipt tail
short. Both graded-span endpoints are fixed overhead: ~0.5us of
framework const-pool memsets up front and a ~6.5us NEFF epilogue that
clears all 254 semaphores at ~150ns each.
"""

from contextlib import ExitStack

import numpy as np

import concourse.tile as tile
from concourse import bacc, mybir
from concourse.bass_utils import run_bass_kernel_spmd

NB_COMP = 8
LAT_DIM = 512
NB_NEUR = 1024
OUT_DIM = 512
N_CORES = 8
KC1, MC1 = LAT_DIM // 128, NB_NEUR // 128  # 4, 8
KC2, MC2 = NB_NEUR // 128, OUT_DIM // 128  # 8, 4

F32 = mybir.dt.float32
BF16 = mybir.dt.bfloat16
NP_BF16 = mybir.dt.np(BF16)
SIG = mybir.ActivationFunctionType.Sigmoid
IDENT = mybir.ActivationFunctionType.Identity

_program_cache = {}


def _make_chunks(k_cap):
    # 512-wide chunks (PSUM-bank limit), remainder split so the final
    # chunk is 128 wide: the end-of-kernel bias-add + store + DMA-receipt
    # tail scales with the last chunk, while extra chunks cost almost
    # nothing (~8ns/matmul dispatch overhead warm).
    chunks = []
    rem = k_cap
    while rem >= 512 + 128:
        chunks.append(512)
        rem -= 512
    if rem > 256:
        chunks.append(rem - 128)
        chunks.append(128)
    elif rem:
        chunks.append(rem)
    out = []
    n0 = 0
    for ns in chunks:
        out.append((n0, ns))
        n0 += ns
    return out


def _build_program(k_cap):
    """One-expert MLP over k_cap samples; same program runs SPMD on all 8 cores."""
    chunks = _make_chunks(k_cap)

    nc = bacc.Bacc(
        "TRN2",
        target_bir_lowering=False,
        debug=False,
        enable_asserts=False,
        num_devices=N_CORES,
    )
    # Host-packed layouts, partition dim first everywhere:
    #   x[p, j, n]  = epsT[j*128 + p, n]       (j = mm1 contraction block)
    #   w1[p, j, m] = W1p[j*128 + p, m]
    #   w2[p, j, o] = W2[j*128 + p, o]
    #   b1[p, mc]   = b1p[mc*128 + p],  b2[p, oc] = b2[oc*128 + p]
    #   y[p, oc, n] = yT[oc*128 + p, n]
    x_d = nc.dram_tensor("x", [128, KC1, k_cap], BF16, kind="ExternalInput").ap()
    w1_d = nc.dram_tensor("w1", [128, KC1, NB_NEUR], BF16, kind="ExternalInput").ap()
    b1_d = nc.dram_tensor("b1", [128, MC1], F32, kind="ExternalInput").ap()
    w2_d = nc.dram_tensor("w2", [128, KC2, OUT_DIM], BF16, kind="ExternalInput").ap()
    b2_d = nc.dram_tensor("b2", [128, MC2], F32, kind="ExternalInput").ap()
    y_d = nc.dram_tensor("y", [128, MC2, k_cap], BF16, kind="ExternalOutput").ap()

    with tile.TileContext(nc) as tc, ExitStack() as ctx:
        wpool = ctx.enter_context(tc.tile_pool(name="weights", bufs=1))
        xpool = ctx.enter_context(tc.tile_pool(name="x", bufs=1))
        hpool = ctx.enter_context(tc.tile_pool(name="h", bufs=1))
        ypool = ctx.enter_context(tc.tile_pool(name="y", bufs=2))
        # One shared pool holding all 8 PSUM banks; mm1 keeps 8 accumulators
        # live (kc-outer order), mm2 4, cycling through the same slots.
        pspool = ctx.enter_context(tc.tile_pool(name="ps", bufs=8, space="PSUM"))

        # Warm the PE's HAM clock gate while the first DMAs are in flight:
        # the gate needs ~3.4us of sustained PE activity to lift the 1.2GHz
        # cold throttle, so a dozen dummy matmuls on a memset tile make the
        # real matmuls run at 2.4GHz from their first instruction.
        warm = wpool.tile([128, 512], BF16, tag="warm")
        nc.vector.memset(warm[:], 0.0)
        wps = pspool.tile([128, 512], F32, tag="ps", name="warm_ps")
        for _ in range(16):
            nc.tensor.matmul(wps[:], warm[:, :128], warm[:], start=True, stop=True)

        # Startup is DMA-supply-bound: the sync HWDGE ring delivers
        # ~275-320 GB/s with ~1.5us start latency and ~1.5us
        # completion-receipt lag per dma_start (the ACT ring measured 2.3x
        # slower per byte - don't use it). The warmup matmul count above is
        # sized so the real matmul stream starts right as this FIFO can
        # sustain it; blocks are ordered by when the PE needs them.
        n0_0, ns_0 = chunks[0]
        w1a = wpool.tile([128, NB_NEUR], BF16, tag="w1a")
        nc.sync.dma_start(w1a[:], w1_d[:, 0, :])
        xt = []
        t = xpool.tile([128, KC1, ns_0], BF16, tag="x0", name="x_0")
        nc.sync.dma_start(t[:], x_d[:, :, n0_0 : n0_0 + ns_0])
        xt.append(t)
        w1b = wpool.tile([128, NB_NEUR], BF16, tag="w1b")
        nc.sync.dma_start(w1b[:], w1_d[:, 1, :])
        b1t = wpool.tile([128, MC1], F32, tag="b1")
        nc.sync.dma_start(b1t[:], b1_d[:])
        w1c = wpool.tile([128, KC1 - 2, NB_NEUR], BF16, tag="w1c")
        nc.sync.dma_start(w1c[:], w1_d[:, 2:, :])
        w2a = wpool.tile([128, 2, OUT_DIM], BF16, tag="w2a")
        nc.sync.dma_start(w2a[:], w2_d[:, :2, :])
        w2b = wpool.tile([128, KC2 - 2, OUT_DIM], BF16, tag="w2b")
        nc.sync.dma_start(w2b[:], w2_d[:, 2:, :])
        b2t = wpool.tile([128, MC2], F32, tag="b2")
        nc.sync.dma_start(b2t[:], b2_d[:])

        def w2_ap(kc, oc):
            if kc < 2:
                return w2a[:, kc, oc * 128 : (oc + 1) * 128]
            return w2b[:, kc - 2, oc * 128 : (oc + 1) * 128]
        if len(chunks) > 1:
            n0, ns = chunks[1]
            t = xpool.tile([128, KC1, ns], BF16, tag="x1", name="x_1")
            nc.sync.dma_start(t[:], x_d[:, :, n0 : n0 + ns])
            xt.append(t)
        if len(chunks) > 2:
            n0r = chunks[2][0]
            nsr = k_cap - n0r
            xr = xpool.tile([128, KC1, nsr], BF16, tag="xr", name="x_rest")
            nc.sync.dma_start(xr[:], x_d[:, :, n0r:])
        else:
            xr = None

        def x_ap(ci, kc):
            n0, ns = chunks[ci]
            if ci < 2:
                return xt[ci][:, kc, :]
            return xr[:, kc, n0 - n0r : n0 - n0r + ns]

        def w1_ap(kc, mc):
            if kc == 0:
                return w1a[:, mc * 128 : (mc + 1) * 128]
            if kc == 1:
                return w1b[:, mc * 128 : (mc + 1) * 128]
            return w1c[:, kc - 2, mc * 128 : (mc + 1) * 128]

        for ci, (n0, ns) in enumerate(chunks):
            ht = []
            ps1 = [
                pspool.tile([128, ns], F32, tag="ps", name=f"ps1_{ci}_{i}")
                for i in range(MC1)
            ]
            for kc in range(KC1):
                for mc in range(MC1):
                    nc.tensor.matmul(
                        ps1[mc][:],
                        w1_ap(kc, mc),
                        x_ap(ci, kc),
                        start=(kc == 0),
                        stop=(kc == KC1 - 1),
                    )
                    if kc == KC1 - 1:
                        h = hpool.tile([128, ns], BF16, tag=f"h{mc}")
                        nc.scalar.activation(
                            h[:], ps1[mc][:], SIG, bias=b1t[:, mc : mc + 1]
                        )
                        ht.append(h)

            yt = ypool.tile([128, MC2, ns], BF16, tag="y", name=f"y_{ci}")
            ps2 = [
                pspool.tile([128, ns], F32, tag="ps", name=f"ps2_{ci}_{i}")
                for i in range(MC2)
            ]
            for kc in range(KC2):
                for oc in range(MC2):
                    nc.tensor.matmul(
                        ps2[oc][:],
                        w2_ap(kc, oc),
                        ht[kc][:],
                        start=(kc == 0),
                        stop=(kc == KC2 - 1),
                    )
                    if kc == KC2 - 1:
                        # Alternate DVE / ACT so the 4 bias-adds of the last
                        # chunk drain on two engines instead of one.
                        if oc % 2 == 0:
                            nc.vector.tensor_scalar_add(
                                yt[:, oc, :], ps2[oc][:], b2t[:, oc : oc + 1]
                            )
                        else:
                            nc.scalar.activation(
                                yt[:, oc, :],
                                ps2[oc][:],
                                IDENT,
                                bias=b2t[:, oc : oc + 1],
                            )
            nc.sync.dma_start(y_d[:, :, n0 : n0 + ns], yt[:])

    nc.compile()
    return nc


def get_program(k_cap):
    if k_cap not in _program_cache:
        _program_cache[k_cap] = _build_program(k_cap)
    return _program_cache[k_cap]


def _softplus(x):
    x = x.astype(np.float64)
    return np.maximum(x, 0.0) + np.log1p(np.exp(-np.abs(x)))


def kernel(epsilon, comp_idx, mu, rho, W1, b1, W2, b2, _trace=False):
    epsilon = np.asarray(epsilon, dtype=np.float32)
    comp_idx = np.asarray(comp_idx, dtype=np.int32)
    mu = np.asarray(mu, dtype=np.float32)
    rho = np.asarray(rho, dtype=np.float32)
    W1 = np.asarray(W1, dtype=np.float32)
    b1 = np.asarray(b1, dtype=np.float32)
    W2 = np.asarray(W2, dtype=np.float32)
    b2 = np.asarray(b2, dtype=np.float32)

    n = epsilon.shape[0]
    sigma = _softplus(rho)  # [C] float64

    sels = [np.nonzero(comp_idx == c)[0] for c in range(NB_COMP)]
    counts = [len(s) for s in sels]
    k_cap = max(128, -(-max(counts) // 8) * 8)

    nc = get_program(k_cap)

    in_maps = []
    for c in range(NB_COMP):
        sel = sels[c]
        x = np.zeros((128, KC1, k_cap), dtype=NP_BF16)
        if len(sel):
            xe = epsilon[sel].T.reshape(KC1, 128, len(sel)).transpose(1, 0, 2)
            x[:, :, : len(sel)] = xe.astype(NP_BF16)
        w1p = (W1[c].astype(np.float64) * sigma[c]).astype(np.float32)
        b1p = b1[c].astype(np.float64) + mu[c].astype(np.float64) @ W1[c].astype(
            np.float64
        )
        in_maps.append(
            {
                "x": x,
                "w1": np.ascontiguousarray(
                    w1p.reshape(KC1, 128, NB_NEUR).transpose(1, 0, 2)
                ).astype(NP_BF16),
                "b1": np.ascontiguousarray(
                    b1p.astype(np.float32).reshape(MC1, 128).T
                ),
                "w2": np.ascontiguousarray(
                    W2[c].reshape(KC2, 128, OUT_DIM).transpose(1, 0, 2)
                ).astype(NP_BF16),
                "b2": np.ascontiguousarray(b2[c].reshape(MC2, 128).T),
            }
        )

    res = run_bass_kernel_spmd(
        nc,
        in_maps,
        core_ids=list(range(N_CORES)),
        trace=_trace,
        trace_cores=list(range(N_CORES)) if _trace else None,
    )

    out = np.zeros((n, OUT_DIM), dtype=np.float32)
    for c in range(NB_COMP):
        sel = sels[c]
        if len(sel):
            y = np.asarray(res.results[c]["y"], dtype=np.float32)  # [128, MC2, k]
            out[sel] = y.transpose(2, 1, 0)[: len(sel)].reshape(len(sel), OUT_DIM)
    if _trace:
        return out, res
    return out


# revision 21
# speedup vs baseline: 1.1247x; 1.0098x over previous
"""GNN ensemble MoE-routing kernel for Trainium2 (8 NeuronCores).

Reference computes all 8 expert MLPs for every sample then selects one
(8x wasted FLOPs). This kernel routes on the host instead: samples are
gathered per expert, and core c runs ONLY expert c's MLP over the
samples routed to it (expert-parallel sharding).

Math folding (exact):
  lat = eps*sigma_c + mu_c  =>  lat @ W1_c = eps @ (sigma_c*W1_c) + mu_c@W1_c
so the device computes  sigmoid(eps @ W1p + b1p) @ W2 + b2  with
  W1p = sigma_c * W1_c,  b1p = b1_c + mu_c @ W1_c  (folded on host).

Device layout: features on SBUF partitions, samples on the free axis.
  epsT [512, K]  ->  H^T [1024, K]  ->  Y^T [512, K]
All matmul operands are bf16 (PSUM accumulation stays fp32; rel-err
~3e-3 vs the 2e-2 gate): 1 PE cycle/column like fp32r but half the HBM
traffic. The host pre-interleaves every tensor into a [128, blocks, n]
layout so each loads in a couple of coalesced dma_starts (a dma_start
costs ~0.6us serialized on the Sync sequencer; the fp32r baseline
spent ~12us just issuing its 54 descriptors, and late x-chunk issues
starved the PE mid-kernel). Startup is supply-bound on the sync HWDGE
ring (~300 GB/s + ~1.5us start + ~1.5us completion-receipt latency),
so dummy warm-up matmuls both lift the PE HAM clock gate (1.2 -> 2.4
GHz after ~3.4us of sustained activity) and absorb the supply latency;
input blocks are FIFO-ordered to land exactly when the matmul stream
reaches them (measured: zero PE gaps). k_cap is the max per-expert
count rounded to 8 (the moving dim needs no 128 alignment), and the
final 128-wide chunk keeps the bias-add + store + DMA-receipt tail
short. Both graded-span endpoints are fixed overhead: ~0.5us of
framework const-pool memsets up front and a ~6.5us NEFF epilogue that
clears all 254 semaphores at ~150ns each.
"""

from contextlib import ExitStack

import numpy as np

import concourse.tile as tile
from concourse import bacc, mybir
from concourse.bass_utils import run_bass_kernel_spmd

NB_COMP = 8
LAT_DIM = 512
NB_NEUR = 1024
OUT_DIM = 512
N_CORES = 8
KC1, MC1 = LAT_DIM // 128, NB_NEUR // 128  # 4, 8
KC2, MC2 = NB_NEUR // 128, OUT_DIM // 128  # 8, 4

F32 = mybir.dt.float32
BF16 = mybir.dt.bfloat16
NP_BF16 = mybir.dt.np(BF16)
SIG = mybir.ActivationFunctionType.Sigmoid
IDENT = mybir.ActivationFunctionType.Identity

_program_cache = {}


def _make_chunks(k_cap):
    # 512-wide chunks (PSUM-bank limit), remainder split so the final
    # chunk is 128 wide: the end-of-kernel bias-add + store + DMA-receipt
    # tail scales with the last chunk, while extra chunks cost almost
    # nothing (~8ns/matmul dispatch overhead warm).
    chunks = []
    rem = k_cap
    while rem >= 512 + 128:
        chunks.append(512)
        rem -= 512
    if rem > 256:
        chunks.append(rem - 128)
        chunks.append(128)
    elif rem:
        chunks.append(rem)
    out = []
    n0 = 0
    for ns in chunks:
        out.append((n0, ns))
        n0 += ns
    return out


def _build_program(k_cap):
    """One-expert MLP over k_cap samples; same program runs SPMD on all 8 cores."""
    chunks = _make_chunks(k_cap)

    nc = bacc.Bacc(
        "TRN2",
        target_bir_lowering=False,
        debug=False,
        enable_asserts=False,
        num_devices=N_CORES,
    )
    # Host-packed layouts, partition dim first everywhere:
    #   x[p, j, n]  = epsT[j*128 + p, n]       (j = mm1 contraction block)
    #   w1[p, j, m] = W1p[j*128 + p, m]
    #   w2[p, j, o] = W2[j*128 + p, o]
    #   b1[p, mc]   = b1p[mc*128 + p],  b2[p, oc] = b2[oc*128 + p]
    #   y[p, oc, n] = yT[oc*128 + p, n]
    x_d = nc.dram_tensor("x", [128, KC1, k_cap], BF16, kind="ExternalInput").ap()
    w1_d = nc.dram_tensor("w1", [128, KC1, NB_NEUR], BF16, kind="ExternalInput").ap()
    b1_d = nc.dram_tensor("b1", [128, MC1], F32, kind="ExternalInput").ap()
    w2_d = nc.dram_tensor("w2", [128, KC2, OUT_DIM], BF16, kind="ExternalInput").ap()
    b2_d = nc.dram_tensor("b2", [128, MC2], F32, kind="ExternalInput").ap()
    y_d = nc.dram_tensor("y", [128, MC2, k_cap], BF16, kind="ExternalOutput").ap()

    with tile.TileContext(nc) as tc, ExitStack() as ctx:
        wpool = ctx.enter_context(tc.tile_pool(name="weights", bufs=1))
        xpool = ctx.enter_context(tc.tile_pool(name="x", bufs=1))
        hpool = ctx.enter_context(tc.tile_pool(name="h", bufs=1))
        ypool = ctx.enter_context(tc.tile_pool(name="y", bufs=2))
        # One shared pool holding all 8 PSUM banks; mm1 keeps 8 accumulators
        # live (kc-outer order), mm2 4, cycling through the same slots.
        pspool = ctx.enter_context(tc.tile_pool(name="ps", bufs=8, space="PSUM"))

        # Warm the PE's HAM clock gate while the first DMAs are in flight:
        # the gate needs ~3.4us of sustained PE activity to lift the 1.2GHz
        # cold throttle, so a dozen dummy matmuls on a memset tile make the
        # real matmuls run at 2.4GHz from their first instruction.
        warm = wpool.tile([128, 512], BF16, tag="warm")
        nc.vector.memset(warm[:], 0.0)
        wps = pspool.tile([128, 512], F32, tag="ps", name="warm_ps")
        for _ in range(11):
            nc.tensor.matmul(wps[:], warm[:, :128], warm[:], start=True, stop=True)

        # Startup is DMA-supply-bound: the sync HWDGE ring delivers
        # ~275-320 GB/s with ~1.5us start latency and ~1.5us
        # completion-receipt lag per dma_start (the ACT ring measured 2.3x
        # slower per byte - don't use it). The warmup matmul count above is
        # sized so the real matmul stream starts right as this FIFO can
        # sustain it; blocks are ordered by when the PE needs them.
        n0_0, ns_0 = chunks[0]
        w1a = wpool.tile([128, NB_NEUR], BF16, tag="w1a")
        nc.sync.dma_start(w1a[:], w1_d[:, 0, :])
        xt = []
        t = xpool.tile([128, KC1, ns_0], BF16, tag="x0", name="x_0")
        nc.sync.dma_start(t[:], x_d[:, :, n0_0 : n0_0 + ns_0])
        xt.append(t)
        w1b = wpool.tile([128, NB_NEUR], BF16, tag="w1b")
        nc.sync.dma_start(w1b[:], w1_d[:, 1, :])
        b1t = wpool.tile([128, MC1], F32, tag="b1")
        nc.sync.dma_start(b1t[:], b1_d[:])
        w1c = wpool.tile([128, KC1 - 2, NB_NEUR], BF16, tag="w1c")
        nc.sync.dma_start(w1c[:], w1_d[:, 2:, :])
        w2a = wpool.tile([128, 2, OUT_DIM], BF16, tag="w2a")
        nc.sync.dma_start(w2a[:], w2_d[:, :2, :])
        w2b = wpool.tile([128, KC2 - 2, OUT_DIM], BF16, tag="w2b")
        nc.sync.dma_start(w2b[:], w2_d[:, 2:, :])
        b2t = wpool.tile([128, MC2], F32, tag="b2")
        nc.sync.dma_start(b2t[:], b2_d[:])

        def w2_ap(kc, oc):
            if kc < 2:
                return w2a[:, kc, oc * 128 : (oc + 1) * 128]
            return w2b[:, kc - 2, oc * 128 : (oc + 1) * 128]
        if len(chunks) > 1:
            n0, ns = chunks[1]
            t = xpool.tile([128, KC1, ns], BF16, tag="x1", name="x_1")
            nc.sync.dma_start(t[:], x_d[:, :, n0 : n0 + ns])
            xt.append(t)
        if len(chunks) > 2:
            n0r = chunks[2][0]
            nsr = k_cap - n0r
            xr = xpool.tile([128, KC1, nsr], BF16, tag="xr", name="x_rest")
            nc.sync.dma_start(xr[:], x_d[:, :, n0r:])
        else:
            xr = None

        def x_ap(ci, kc):
            n0, ns = chunks[ci]
            if ci < 2:
                return xt[ci][:, kc, :]
            return xr[:, kc, n0 - n0r : n0 - n0r + ns]

        def w1_ap(kc, mc):
            if kc == 0:
                return w1a[:, mc * 128 : (mc + 1) * 128]
            if kc == 1:
                return w1b[:, mc * 128 : (mc + 1) * 128]
            return w1c[:, kc - 2, mc * 128 : (mc + 1) * 128]

        for ci, (n0, ns) in enumerate(chunks):
            ht = []
            ps1 = [
                pspool.tile([128, ns], F32, tag="ps", name=f"ps1_{ci}_{i}")
                for i in range(MC1)
            ]
            for kc in range(KC1):
                for mc in range(MC1):
                    nc.tensor.matmul(
                        ps1[mc][:],
                        w1_ap(kc, mc),
                        x_ap(ci, kc),
                        start=(kc == 0),
                        stop=(kc == KC1 - 1),
                    )
                    if kc == KC1 - 1:
                        h = hpool.tile([128, ns], BF16, tag=f"h{mc}")
                        nc.scalar.activation(
                            h[:], ps1[mc][:], SIG, bias=b1t[:, mc : mc + 1]
                        )
                        ht.append(h)

            yt = ypool.tile([128, MC2, ns], BF16, tag="y", name=f"y_{ci}")
            ps2 = [
                pspool.tile([128, ns], F32, tag="ps", name=f"ps2_{ci}_{i}")
                for i in range(MC2)
            ]
            for kc in range(KC2):
                for oc in range(MC2):
                    nc.tensor.matmul(
                        ps2[oc][:],
                        w2_ap(kc, oc),
                        ht[kc][:],
                        start=(kc == 0),
                        stop=(kc == KC2 - 1),
                    )
                    if kc == KC2 - 1:
                        # Alternate DVE / ACT so the 4 bias-adds of the last
                        # chunk drain on two engines instead of one.
                        if oc % 2 == 0:
                            nc.vector.tensor_scalar_add(
                                yt[:, oc, :], ps2[oc][:], b2t[:, oc : oc + 1]
                            )
                        else:
                            nc.scalar.activation(
                                yt[:, oc, :],
                                ps2[oc][:],
                                IDENT,
                                bias=b2t[:, oc : oc + 1],
                            )
            nc.sync.dma_start(y_d[:, :, n0 : n0 + ns], yt[:])

    nc.compile()
    return nc


def get_program(k_cap):
    if k_cap not in _program_cache:
        _program_cache[k_cap] = _build_program(k_cap)
    return _program_cache[k_cap]


def _softplus(x):
    x = x.astype(np.float64)
    return np.maximum(x, 0.0) + np.log1p(np.exp(-np.abs(x)))


def kernel(epsilon, comp_idx, mu, rho, W1, b1, W2, b2, _trace=False):
    epsilon = np.asarray(epsilon, dtype=np.float32)
    comp_idx = np.asarray(comp_idx, dtype=np.int32)
    mu = np.asarray(mu, dtype=np.float32)
    rho = np.asarray(rho, dtype=np.float32)
    W1 = np.asarray(W1, dtype=np.float32)
    b1 = np.asarray(b1, dtype=np.float32)
    W2 = np.asarray(W2, dtype=np.float32)
    b2 = np.asarray(b2, dtype=np.float32)

    n = epsilon.shape[0]
    sigma = _softplus(rho)  # [C] float64

    sels = [np.nonzero(comp_idx == c)[0] for c in range(NB_COMP)]
    counts = [len(s) for s in sels]
    k_cap = max(128, -(-max(counts) // 8) * 8)

    nc = get_program(k_cap)

    in_maps = []
    for c in range(NB_COMP):
        sel = sels[c]
        x = np.zeros((128, KC1, k_cap), dtype=NP_BF16)
        if len(sel):
            xe = epsilon[sel].T.reshape(KC1, 128, len(sel)).transpose(1, 0, 2)
            x[:, :, : len(sel)] = xe.astype(NP_BF16)
        w1p = (W1[c].astype(np.float64) * sigma[c]).astype(np.float32)
        b1p = b1[c].astype(np.float64) + mu[c].astype(np.float64) @ W1[c].astype(
            np.float64
        )
        in_maps.append(
            {
                "x": x,
                "w1": np.ascontiguousarray(
                    w1p.reshape(KC1, 128, NB_NEUR).transpose(1, 0, 2)
                ).astype(NP_BF16),
                "b1": np.ascontiguousarray(
                    b1p.astype(np.float32).reshape(MC1, 128).T
                ),
                "w2": np.ascontiguousarray(
                    W2[c].reshape(KC2, 128, OUT_DIM).transpose(1, 0, 2)
                ).astype(NP_BF16),
                "b2": np.ascontiguousarray(b2[c].reshape(MC2, 128).T),
            }
        )

    res = run_bass_kernel_spmd(
        nc,
        in_maps,
        core_ids=list(range(N_CORES)),
        trace=_trace,
        trace_cores=list(range(N_CORES)) if _trace else None,
    )

    out = np.zeros((n, OUT_DIM), dtype=np.float32)
    for c in range(NB_COMP):
        sel = sels[c]
        if len(sel):
            y = np.asarray(res.results[c]["y"], dtype=np.float32)  # [128, MC2, k]
            out[sel] = y.transpose(2, 1, 0)[: len(sel)].reshape(len(sel), OUT_DIM)
    if _trace:
        return out, res
    return out
